# revision 1
# baseline (speedup 1.0000x reference)
"""Causal self-attention (B=2, N=4096, C=768, 12 heads, d=64) on 8 trn2 cores.

Sharding: core (b, g) = batch b, head-group g (3 heads). Tensor-parallel on
heads: each core computes qkv projection for its 3 heads, causal flash
attention, and a partial output projection; host sums the 4 partials per batch
and adds b_out.

Device layout notes:
 - All matmuls bf16 (fp32 PSUM accumulation).
 - Qt/Kt produced directly in [d, seq] layout by using W as the stationary
   matmul operand against host-pretransposed xT.
 - Scores St in [k, q] layout; probs = exp(St/8) with NO max subtraction
   (scores are bounded ~|2|), causal mask applied as a 0/1 bf16 multiply on
   diagonal blocks only.
 - PV: stationary [V_h | ones | 0] (66 cols) -> O^T rows 0-63, softmax
   denominator in row 64 for free.
 - Weight-group layout [q0|q1][k0|k1][q2|q2][k2|k2] lets QK^T run pairs of
   K=64 matmuls concurrently on disjoint PE row groups.
"""

import numpy as np
import ml_dtypes

import concourse.bass as bass
import concourse.mybir as mybir
import concourse.tile as tile
from concourse import bass_utils
from concourse.vector_clock import ScopedClock

P = 128
D = 64
C = 768
HL = 3          # heads per core
QT = 512        # q tile width
VW = 66 * HL    # v sbuf row width: [v_h(64) | ones | zero] x 3
N_CORES = 8
BF = mybir.dt.bfloat16
F32 = mybir.dt.float32
BF_NP = ml_dtypes.bfloat16


class PatchedTileContext(tile.TileContext):
    """This toolchain's walrus rejects more than ONE sync-wait on any
    instruction ("Too many sync wait commands"). Tile's wait assignment
    freely attaches several. Legalize: for every instruction with k>1
    waits, insert k-1 same-engine NOPs before it, one wait each."""

    def _split_sync_waits(self):
        nc = self.nc
        for bb in nc.m.functions[0].blocks:
            insts = bb.instructions
            out = []
            changed = False
            for inst in insts:
                si = inst.sync_info
                waits = list(si.on_wait or []) if si is not None else []
                if len(waits) > 1:
                    changed = True
                    for w in waits[:-1]:
                        nop = mybir.InstNoOp(
                            name=f"I-wsplit{nc.next_id()}", text_hint="wsplit")
                        nop.engine = inst.engine
                        nop.sync_info = mybir.SyncInfo(on_wait=[w], on_update=[])
                        nc.register_instruction(nop)
                        out.append(nop)
                    si.on_wait = waits[-1:]
                out.append(inst)
            if changed:
                bb.instructions = out

    def _drain_and_barrier(self, tick_clock, wait_clock):
        drain_inst = self.nc.sync.drain()
        wait_clock.add_sem_waits(
            drain_inst.ins, ScopedClock({None: tick_clock.global_clock})
        )
        si = drain_inst.ins.sync_info
        waits = list(si.on_wait or []) if si is not None else []
        if len(waits) > 1:
            si.on_wait = waits[:1]
            for w in waits[1:]:
                extra = self.nc.sync.drain()
                esi = extra.ins.sync_info
                if esi is None:
                    extra.ins.sync_info = mybir.SyncInfo(on_wait=[w], on_update=[])
                else:
                    esi.on_wait = [w]

        self.nc.all_engine_barrier()
        assert self.sems is not None
        popped = self.nc._tile_sem_poison_stack.pop()
        assert popped is self._sem_poison
        # clear_and_free_semaphores would emit EVENT_SEMAPHORE_RANGE_CLEAR
        # (an InstISA), which this walrus rejects ("ISA wrong length") — and
        # per-sem sem_clear lowers to the same opcode. Skip the clears: this
        # is the only TileContext in the NEFF and NRT re-initializes
        # semaphores per execution (verified empirically by repeated runs).
        self.nc.all_engine_barrier()
        self._split_sync_waits()


def build_nc(n_seq=4096):
    CC = C // P                  # 6 contraction chunks
    NQ = n_seq // QT             # q tiles
    nc = bass.Bass("TRN2", target_bir_lowering=False, debug=False,
                   num_devices=N_CORES)

    xT = nc.dram_tensor("xT", [C, n_seq], BF, kind="ExternalInput").ap()
    wqk = nc.dram_tensor("wqk", [C, 4 * P], BF, kind="ExternalInput").ap()
    bqk = nc.dram_tensor("bqk", [P, 4], F32, kind="ExternalInput").ap()
    wv = nc.dram_tensor("wv", [C, VW], BF, kind="ExternalInput").ap()
    bv = nc.dram_tensor("bv", [P, VW], F32, kind="ExternalInput").ap()
    wo = nc.dram_tensor("wo", [D, HL, C], BF, kind="ExternalInput").ap()
    mask = nc.dram_tensor("mask", [P, 4, QT], BF, kind="ExternalInput").ap()
    out = nc.dram_tensor("out", [n_seq, C], F32, kind="ExternalOutput").ap()

    Exp = mybir.ActivationFunctionType.Exp

    from contextlib import ExitStack
    with PatchedTileContext(nc) as tc, ExitStack() as ctx:
        consts = ctx.enter_context(tc.tile_pool(name="consts", bufs=1))
        # weights / constants
        wqk_sb = consts.tile([P, CC, 4 * P], BF, name="wqk_sb")
        nc.sync.dma_start(wqk_sb[:], wqk.rearrange("(o p) m -> p o m", p=P))
        bqk_sb = consts.tile([P, 4], F32, name="bqk_sb")
        nc.sync.dma_start(bqk_sb[:], bqk[:])
        wv_sb = consts.tile([P, CC, VW], BF, name="wv_sb")
        nc.sync.dma_start(wv_sb[:], wv.rearrange("(o p) m -> p o m", p=P))
        bv_sb = consts.tile([P, VW], F32, name="bv_sb")
        nc.sync.dma_start(bv_sb[:], bv[:])
        wo_sb = consts.tile([D, HL, C], BF, name="wo_sb")
        nc.sync.dma_start(wo_sb[:], wo[:])
        mask_sb = consts.tile([P, 4, QT], BF, name="mask_sb")
        nc.sync.dma_start(mask_sb[:], mask[:])
        xt_sb = []
        for c in range(CC):
            t = consts.tile([P, n_seq], BF, name=f"xt{c}")
            nc.sync.dma_start(t[:], xT[c * P:(c + 1) * P, :])
            xt_sb.append(t)
        # persistent intermediates
        qkt = [[consts.tile([P, QT], BF, name=f"qkt{g}_{s}") for s in range(NQ)]
               for g in range(4)]
        v_s = [consts.tile([P, VW], BF, name=f"v{s}") for s in range(4 * NQ)]
        ot = [consts.tile([D, n_seq], BF, name=f"ot{h}") for h in range(HL)]

        ps_gen = ctx.enter_context(tc.tile_pool(name="ps_gen", bufs=2, space="PSUM"))
        ps_st = ctx.enter_context(tc.tile_pool(name="ps_st", bufs=2, space="PSUM"))
        ps_pv = ctx.enter_context(tc.tile_pool(name="ps_pv", bufs=2, space="PSUM"))
        sb_pt = ctx.enter_context(tc.tile_pool(name="sb_pt", bufs=6))
        sb_nrm = ctx.enter_context(tc.tile_pool(name="sb_nrm", bufs=3))
        sb_out = ctx.enter_context(tc.tile_pool(name="sb_out", bufs=3))
        dr_nrm = ctx.enter_context(tc.tile_pool(name="dr_nrm", bufs=4, space="DRAM"))

        for s in range(NQ):
            qsl = slice(QT * s, QT * (s + 1))
            # ---- qkv projection for this seq tile ----
            for g in range(4):
                ps = ps_gen.tile([P, QT], F32, tag="gen", name=f"psqk{g}_{s}")
                for c in range(CC):
                    nc.tensor.matmul(
                        ps[:], wqk_sb[:, c, P * g:P * (g + 1)],
                        xt_sb[c][:, qsl],
                        start=(c == 0), stop=(c == CC - 1))
                nc.vector.tensor_add(qkt[g][s][:], ps[:],
                                     bqk_sb[:, g:g + 1].to_broadcast((P, QT)))
            for sc in range(4 * s, 4 * s + 4):
                ps = ps_gen.tile([P, QT], F32, tag="gen", name=f"psv{sc}")
                for c in range(CC):
                    nc.tensor.matmul(
                        ps[:, :VW], xt_sb[c][:, P * sc:P * (sc + 1)],
                        wv_sb[:, c, :],
                        start=(c == 0), stop=(c == CC - 1))
                nc.vector.tensor_add(v_s[sc][:], ps[:, :VW], bv_sb[:])

            # ---- attention for q-tile j = s ----
            j = s
            nkc = 4 * (j + 1)          # causal k chunks
            # heads 0,1: one St tile per k chunk, slots = heads (row-paired)
            pv01 = [ps_pv.tile([D + 2, QT], F32, tag="pv", name=f"pv{j}_{h}")
                    for h in range(2)]
            for kc in range(nkc):
                si, co = kc // 4, P * (kc % 4)
                # causal: columns below off are fully masked for this chunk —
                # skip them in QK^T and exp (stale psum there is never read),
                # zero them in pt, and mask only the triangular boundary.
                off = P * (kc - 4 * j) if kc >= 4 * j else 0
                stp = ps_st.tile([P, 2, QT], F32, tag="st", name=f"st{j}_{kc}")
                for h in range(2):
                    r = slice(D * h, D * (h + 1))
                    nc.tensor.matmul(stp[:, h, off:],
                                     qkt[1][si][r, co:co + P],
                                     qkt[0][j][r, off:],
                                     start=True, stop=True)
                pt = sb_pt.tile([P, 2, QT], BF, tag="pt", name=f"pt{j}_{kc}")
                if off:
                    nc.vector.memset(pt[:, :, :off], 0.0)
                nc.scalar.activation(pt[:, :, off:], stp[:, :, off:],
                                     Exp, scale=0.125)
                if kc >= 4 * j:
                    m = mask_sb[:, kc - 4 * j, off:off + P]
                    for h in range(2):
                        nc.vector.tensor_mul(pt[:, h, off:off + P],
                                             pt[:, h, off:off + P], m)
                for h in range(2):
                    nc.tensor.matmul(pv01[h][:],
                                     v_s[kc][:, 66 * h:66 * h + 66],
                                     pt[:, h, :],
                                     start=(kc == 0), stop=(kc == nkc - 1))
            # head 2: slots = consecutive k chunks (self row-paired)
            pv2 = ps_pv.tile([D + 2, QT], F32, tag="pv", name=f"pv{j}_2")
            for gk in range(nkc // 2):
                st2 = ps_st.tile([P, 2, QT], F32, tag="st", name=f"st2_{j}_{gk}")
                for cr in range(2):
                    kc = 2 * gk + cr
                    si, co = kc // 4, P * (kc % 4)
                    r = slice(D * (kc % 2), D * (kc % 2) + D)
                    nc.tensor.matmul(st2[:, cr, :],
                                     qkt[3][si][r, co:co + P],
                                     qkt[2][j][r, :],
                                     start=True, stop=True)
                pt = sb_pt.tile([P, 2, QT], BF, tag="pt", name=f"pt2_{j}_{gk}")
                nc.scalar.activation(pt[:], st2[:], Exp, scale=0.125)
                if gk >= 2 * j:
                    for cr in range(2):
                        m = mask_sb[:, 2 * (gk - 2 * j) + cr, :]
                        nc.vector.tensor_mul(pt[:, cr, :], pt[:, cr, :], m)
                for cr in range(2):
                    kc = 2 * gk + cr
                    nc.tensor.matmul(pv2[:],
                                     v_s[kc][:, 66 * 2:66 * 2 + 66],
                                     pt[:, cr, :],
                                     start=(kc == 0), stop=(kc == nkc - 1))
            # normalize O^T by the denominators (psum row 64): batched per-j
            # chain — den rows -> DRAM -> [128, 3*QT/128] so the exact 6-cpe
            # DVE reciprocal uses all lanes, then bounce back replicated.
            # Bounce DMAs ride the idle Pool engine's SWDGE so the dependent
            # chain never blocks SP's bulk DMA queue.
            for h, pvp in enumerate(pv01 + [pv2]):
                # stage O^T and den out of PSUM immediately: frees the pv
                # psum slot after two quick DVE copies, so the bounce chain
                # below never gates the next q-tile's PV matmuls.
                osg = sb_nrm.tile([D, QT], F32, tag="osg", name=f"osg{j}_{h}")
                nc.vector.tensor_copy(osg[:], pvp[0:D, :])
                den = sb_nrm.tile([P, QT], F32, tag="den", name=f"den{j}_{h}")
                nc.vector.tensor_copy(den[D:D + 1, :], pvp[D:D + 1, :])
                scr = dr_nrm.tile([QT], F32, tag="scr", name=f"scr{j}_{h}")
                nc.gpsimd.dma_start(scr[None, :], den[D:D + 1, :])
                dfold = sb_nrm.tile([P, QT // P], F32, tag="dfold",
                                    name=f"dfold{j}_{h}")
                nc.gpsimd.dma_start(dfold[:],
                                    scr.rearrange("(p f) -> p f", p=P))
                rfold = sb_nrm.tile([P, QT // P], F32, tag="rfold",
                                    name=f"rfold{j}_{h}")
                nc.vector.reciprocal(rfold[:], dfold[:])
                scr2 = dr_nrm.tile([QT], F32, tag="scr2", name=f"scr2{j}_{h}")
                nc.gpsimd.dma_start(scr2.rearrange("(p f) -> p f", p=P),
                                    rfold[:])
                rep = sb_nrm.tile([D, QT], F32, tag="rep", name=f"rep{j}_{h}")
                nc.gpsimd.dma_start(rep[:],
                                    scr2[None, :].to_broadcast((D, QT)))
                nc.vector.tensor_mul(ot[h][:, qsl], osg[:], rep[:])

            # ---- output projection, one iteration behind (keeps the norm
            # chain latency off the critical path) ----
            for jp in ([j - 1] if j > 0 else []) + ([j] if j == NQ - 1 else []):
                for qc in range(4 * jp, 4 * jp + 4):
                    osb = sb_out.tile([P, C], F32, tag="osb", name=f"osb{qc}")
                    for nh in range(2):
                        pj = ps_gen.tile([P, QT], F32, tag="gen",
                                         name=f"pj{qc}_{nh}")
                        nsl = slice(384 * nh, 384 * (nh + 1))
                        for h in range(HL):
                            nc.tensor.matmul(pj[:, :384],
                                             ot[h][:, P * qc:P * (qc + 1)],
                                             wo_sb[:, h, nsl],
                                             start=(h == 0), stop=(h == HL - 1))
                        nc.vector.tensor_copy(osb[:, nsl], pj[:, :384])
                    nc.sync.dma_start(out[P * qc:P * (qc + 1), :], osb[:])

    return nc


def make_mask():
    p = np.arange(P)[:, None, None]
    c = np.arange(4)[None, :, None]
    qf = np.arange(QT)[None, None, :]
    return (qf >= P * c + p).astype(BF_NP)


def prep_core_inputs(x, W_attn, b_attn, W_out, b, g, mask):
    """Host-side shard prep for core (batch b, head group g)."""
    habs = [HL * g + h for h in range(HL)]
    wq = [W_attn[:, D * h:D * (h + 1)] for h in habs]
    wk = [W_attn[:, C + D * h:C + D * (h + 1)] for h in habs]
    wv_ = [W_attn[:, 2 * C + D * h:2 * C + D * (h + 1)] for h in habs]
    bq = [b_attn[D * h:D * (h + 1)] for h in habs]
    bk = [b_attn[C + D * h:C + D * (h + 1)] for h in habs]
    bvv = [b_attn[2 * C + D * h:2 * C + D * (h + 1)] for h in habs]

    wqk = np.concatenate(
        [wq[0], wq[1], wk[0], wk[1], wq[2], wq[2], wk[2], wk[2]], axis=1)
    bqk = np.stack([
        np.concatenate([bq[0], bq[1]]),
        np.concatenate([bk[0], bk[1]]),
        np.concatenate([bq[2], bq[2]]),
        np.concatenate([bk[2], bk[2]]),
    ], axis=1).astype(np.float32)

    wv_ext = np.zeros((C, VW), dtype=np.float32)
    bv_ext = np.zeros(VW, dtype=np.float32)
    for h in range(HL):
        wv_ext[:, 66 * h:66 * h + D] = wv_[h]
        bv_ext[66 * h:66 * h + D] = bvv[h]
        bv_ext[66 * h + D] = 1.0
    bv_tile = np.ascontiguousarray(
        np.broadcast_to(bv_ext, (P, VW))).astype(np.float32)

    wo = np.ascontiguousarray(
        W_out[192 * g:192 * (g + 1), :].reshape(HL, D, C).transpose(1, 0, 2))

    return {
        "xT": np.ascontiguousarray(x[b].T).astype(BF_NP),
        "wqk": wqk.astype(BF_NP),
        "bqk": bqk,
        "wv": wv_ext.astype(BF_NP),
        "bv": bv_tile,
        "wo": wo.astype(BF_NP),
        "mask": mask,
    }


_NC_CACHE = {}


def kernel(x, W_attn, b_attn, W_out, b_out):
    x = np.asarray(x, dtype=np.float32)
    W_attn = np.asarray(W_attn, dtype=np.float32)
    b_attn = np.asarray(b_attn, dtype=np.float32)
    W_out = np.asarray(W_out, dtype=np.float32)
    b_out = np.asarray(b_out, dtype=np.float32)
    B, n_seq, _ = x.shape

    if n_seq not in _NC_CACHE:
        _NC_CACHE[n_seq] = build_nc(n_seq)
    nc = _NC_CACHE[n_seq]

    mask = make_mask()
    in_maps = [prep_core_inputs(x, W_attn, b_attn, W_out, b, g, mask)
               for b in range(B) for g in range(4)]
    res = bass_utils.run_bass_kernel_spmd(
        nc, in_maps, core_ids=list(range(N_CORES)))
    parts = [r["out"] for r in res.results]
    out = np.empty((B, n_seq, C), dtype=np.float32)
    for b in range(B):
        out[b] = parts[4 * b] + parts[4 * b + 1] + parts[4 * b + 2] \
            + parts[4 * b + 3] + b_out
    return out



# revision 57
# speedup vs baseline: 1.5411x; 1.5411x over previous
"""Causal self-attention (B=2, N=4096, C=768, 12 heads, d=64) on 8 trn2 cores.

Sharding: core (b, g) = batch b, head-group g (3 heads). Each core computes
the qkv projection for its 3 heads, causal attention, and a partial output
projection; host sums the 4 partials per batch and adds b_out.

Schedule (per core), designed against the TimelineSim cost model where a
matmul costs out_free_size x cycles_per_row (0.5 for fp8 DoubleRow, no
stationary-load cost) and engine ops cost free_size x cycle + fixed init:
 - qkv gen: bf16 matmuls, stationary W [128c,128], moving xT [128c, 512].
   Weight-column packing [q0|q1][k0|k1][q2|q2][k2|k2] (head2 duplicated so
   every QK^T operand pair shares a partition base).
 - q,k stored fp8e4; QK^T via fp8 DoubleRow with BOTH k-tile slabs aliased
   to the same data (stride-0 slab dim) -> scores come out exactly 2x,
   folded into the exp scale. 0.5 cycles/row halves QK cost vs bf16.
 - exp split across Act (native Exp; j<=1 pinned there for accuracy since
   short rows average away less noise) and DVE/Pool (Schraudolph bf16 bit
   trick: u16 = round(s2*0.0625*log2e*128 + 16198.4), bitcast bf16; ~3% rms
   multiplicative noise, washed out by softmax averaging on long rows),
   greedy-balanced by modeled engine busy time.
 - PV in [q, d] form: stationary probs [128kpos, 128q] (free), moving V_ext
   [128, 65] ([v | ones] -> denominator lands in column 64). The PSUM bank
   is memset once per (j, head) and all PV matmuls accumulate with
   start=False (a zero-region is a whole bank, so four proper accumulation
   groups can't share one).
 - normalize per qc chunk with reciprocal of column 64 (per-partition
   scalar -- no cross-partition denominator bounce needed in this layout),
   then PE-transpose o [128q, 64d] -> o^T [64, 128] for the projection.
 - out projection bf16: lhsT = o^T chunks (K=128 packed h0h1 + K=64 h2),
   moving W_out [., 768]; partial written f32 and summed on host.
"""

import numpy as np
import ml_dtypes

import concourse.bass as bass
import concourse.mybir as mybir
import concourse.tile as tile
from concourse import bass_utils
from concourse.vector_clock import ScopedClock

P = 128
D = 64
C = 768
HL = 3          # heads per core
QT = 512        # q tile width
VW = 66 * HL    # v sbuf row width: [v_h(64) | ones | pad] x 3
N_CORES = 8
BF = mybir.dt.bfloat16
F32 = mybir.dt.float32
FP8 = mybir.dt.float8e4
U16 = mybir.dt.uint16
BF_NP = ml_dtypes.bfloat16
FP8_NP = ml_dtypes.float8_e4m3

LOG2E = 1.4426950408889634
# scores arrive 2x-scaled (aliased DoubleRow slabs); exp(s2*0.0625) as
# 2^(s2*0.0625*log2e) via bf16 bit trick with magic bias 16256 - 0.45*128
BIT_SCALE = 0.0625 * LOG2E * 128.0
BIT_BIAS = 16256.0 - 57.6
EXP_HALVES = False
DR = mybir.MatmulPerfMode.DoubleRow
Exp = mybir.ActivationFunctionType.Exp
Identity = mybir.ActivationFunctionType.Identity
Copy = mybir.ActivationFunctionType.Copy
MUL = mybir.AluOpType.mult
ADD = mybir.AluOpType.add


class PatchedTileContext(tile.TileContext):
    """This toolchain's walrus rejects more than ONE sync-wait on any
    instruction ("Too many sync wait commands"). Tile's wait assignment
    freely attaches several. Legalize: for every instruction with k>1
    waits, insert k-1 same-engine NOPs before it, one wait each."""

    def _split_sync_waits(self):
        nc = self.nc
        for bb in nc.m.functions[0].blocks:
            insts = bb.instructions
            out = []
            changed = False
            for inst in insts:
                si = inst.sync_info
                waits = list(si.on_wait or []) if si is not None else []
                if len(waits) > 1:
                    changed = True
                    for w in waits[:-1]:
                        nop = mybir.InstNoOp(
                            name=f"I-wsplit{nc.next_id()}", text_hint="wsplit")
                        nop.engine = inst.engine
                        nop.sync_info = mybir.SyncInfo(on_wait=[w], on_update=[])
                        nc.register_instruction(nop)
                        out.append(nop)
                    si.on_wait = waits[-1:]
                out.append(inst)
            if changed:
                bb.instructions = out

    def _drain_and_barrier(self, tick_clock, wait_clock):
        drain_inst = self.nc.sync.drain()
        wait_clock.add_sem_waits(
            drain_inst.ins, ScopedClock({None: tick_clock.global_clock})
        )
        si = drain_inst.ins.sync_info
        waits = list(si.on_wait or []) if si is not None else []
        if len(waits) > 1:
            si.on_wait = waits[:1]
            for w in waits[1:]:
                extra = self.nc.sync.drain()
                esi = extra.ins.sync_info
                if esi is None:
                    extra.ins.sync_info = mybir.SyncInfo(on_wait=[w], on_update=[])
                else:
                    esi.on_wait = [w]

        self.nc.all_engine_barrier()
        assert self.sems is not None
        popped = self.nc._tile_sem_poison_stack.pop()
        assert popped is self._sem_poison
        # clear_and_free_semaphores would emit EVENT_SEMAPHORE_RANGE_CLEAR
        # (an InstISA), which this walrus rejects ("ISA wrong length") — and
        # per-sem sem_clear lowers to the same opcode. Skip the clears: this
        # is the only TileContext in the NEFF and NRT re-initializes
        # semaphores per execution (verified empirically by repeated runs).
        self.nc.all_engine_barrier()
        self._split_sync_waits()


class Balancer:
    """Greedy engine picker by modeled cumulative busy-ns."""

    POOL_EFF = {"memset": 1.0, "tt": 0.42, "ts": 0.6, "copy": 0.6}

    def __init__(self):
        self.busy = {"act": 0.0, "dve": 0.0, "pool": 0.0}

    @classmethod
    def cost(cls, eng, units, kind="copy"):
        if eng == "act":
            return (units + 222) * 0.833
        if eng == "dve":
            return (units + 120) * 1.04
        return units * 0.833 / cls.POOL_EFF[kind] + 120  # pool

    def pick(self, units, kind="copy", allowed=("act", "dve")):
        # NB: gpsimd (pool) cannot access PSUM on this target, and every
        # elementwise op here reads PSUM — so only act/dve are usable.
        eng = min(allowed, key=lambda e: self.busy[e] + self.cost(e, units, kind))
        self.busy[eng] += self.cost(eng, units, kind)
        return eng

    def charge(self, eng, units, kind="copy"):
        self.busy[eng] += self.cost(eng, units, kind)


def build_nc(n_seq=4096):
    CC = C // P                  # 6 contraction chunks
    NQ = n_seq // QT             # q tiles
    nc = bass.Bass("TRN2", target_bir_lowering=False, debug=False,
                   num_devices=N_CORES)

    xT = nc.dram_tensor("xT", [C, n_seq], BF, kind="ExternalInput").ap()
    wqk = nc.dram_tensor("wqk", [C, 3 * P], BF, kind="ExternalInput").ap()
    bqk = nc.dram_tensor("bqk", [P, 3], F32, kind="ExternalInput").ap()
    wv = nc.dram_tensor("wv", [C, VW], BF, kind="ExternalInput").ap()
    bv = nc.dram_tensor("bv", [P, VW], F32, kind="ExternalInput").ap()
    wo01 = nc.dram_tensor("wo01", [P, C], BF, kind="ExternalInput").ap()
    wo2 = nc.dram_tensor("wo2", [D, C], BF, kind="ExternalInput").ap()
    mask = nc.dram_tensor("mask", [P, P], BF, kind="ExternalInput").ap()
    ident = nc.dram_tensor("ident", [P, P], BF, kind="ExternalInput").ap()
    out = nc.dram_tensor("out", [n_seq, C], F32, kind="ExternalOutput").ap()

    bal = Balancer()

    from contextlib import ExitStack
    with PatchedTileContext(nc) as tc, ExitStack() as ctx:
        consts = ctx.enter_context(tc.tile_pool(name="consts", bufs=1))
        # DMA order: what the first gen group needs comes first (xT row-tile
        # 0 pieces + first wqk column group), then the rest of the weights,
        # then the remaining xT pieces s-major.
        xt_sb = [consts.tile([P, n_seq], BF, name=f"xt{c}") for c in range(CC)]
        wqk_sb = consts.tile([P, CC, 3 * P], BF, name="wqk_sb")
        wqk_r = wqk.rearrange("(o p) m -> p o m", p=P)
        for c in range(CC):
            nc.sync.dma_start(xt_sb[c][:, 0:QT], xT[c * P:(c + 1) * P, 0:QT])
        for g in range(3):
            nc.sync.dma_start(wqk_sb[:, :, P * g:P * (g + 1)],
                              wqk_r[:, :, P * g:P * (g + 1)])
        bqk_sb = consts.tile([P, 3], F32, name="bqk_sb")
        nc.sync.dma_start(bqk_sb[:], bqk[:])
        wv_sb = consts.tile([P, CC, VW], BF, name="wv_sb")
        nc.sync.dma_start(wv_sb[:], wv.rearrange("(o p) m -> p o m", p=P))
        bv_sb = consts.tile([P, VW], F32, name="bv_sb")
        nc.sync.dma_start(bv_sb[:], bv[:])
        wo01_sb = consts.tile([P, C], BF, name="wo01_sb")
        nc.sync.dma_start(wo01_sb[:], wo01[:])
        wo2_sb = consts.tile([D, C], BF, name="wo2_sb")
        nc.sync.dma_start(wo2_sb[:], wo2[:])
        mask_sb = consts.tile([P, P], BF, name="mask_sb")
        nc.sync.dma_start(mask_sb[:], mask[:])
        id_sb = consts.tile([P, P], BF, name="id_sb")
        nc.sync.dma_start(id_sb[:], ident[:])
        for s in range(1, NQ):
            for c in range(CC):
                nc.sync.dma_start(xt_sb[c][:, QT * s:QT * (s + 1)],
                                  xT[c * P:(c + 1) * P, QT * s:QT * (s + 1)])
        # persistent intermediates: [q0|q1], [k0|k1], [q2|k2], and a dup
        # tile whose bottom half receives q2 via SBUF->SBUF DMA so head2's
        # QK operands share partition base 64
        qk8 = [consts.tile([P, n_seq], FP8, name=f"qk8_{g}") for g in range(3)]
        q2b = consts.tile([P, n_seq], FP8, name="q2b")
        v_s = [consts.tile([P, VW], BF, name=f"v{sc}") for sc in range(4 * NQ)]
        ot01 = consts.tile([P, n_seq], BF, name="ot01")
        ot2 = consts.tile([D, n_seq], BF, name="ot2")

        ps_gen = ctx.enter_context(tc.tile_pool(name="ps_gen", bufs=1, space="PSUM"))
        ps_st = ctx.enter_context(tc.tile_pool(name="ps_st", bufs=4, space="PSUM"))
        ps_pv = ctx.enter_context(tc.tile_pool(name="ps_pv", bufs=1, space="PSUM"))
        ps_tp = ctx.enter_context(tc.tile_pool(name="ps_tp", bufs=1, space="PSUM"))
        ps_pj = ctx.enter_context(tc.tile_pool(name="ps_pj", bufs=1, space="PSUM"))
        sb_pt = ctx.enter_context(tc.tile_pool(name="sb_pt", bufs=9))
        sb_on = ctx.enter_context(tc.tile_pool(name="sb_on", bufs=4))
        sb_dn = ctx.enter_context(tc.tile_pool(name="sb_dn", bufs=3))
        sb_out = ctx.enter_context(tc.tile_pool(name="sb_out", bufs=3))

        def eng_copy(eng, dst, src):
            if eng == "act":
                nc.scalar.activation(dst, src, Copy)
            elif eng == "dve":
                nc.vector.tensor_copy(dst, src)
            else:
                nc.gpsimd.tensor_copy(dst, src)

        def copy_qk(ps, g, qsl):
            """PSUM f32 + per-partition bias -> fp8 q/k tile."""
            eng = bal.pick(QT, "tt")
            dst = qk8[g][:, qsl]
            b = bqk_sb[:, g:g + 1]
            if eng == "act":
                nc.scalar.activation(dst, ps[:], Identity, bias=b, scale=1.0)
            elif eng == "dve":
                nc.vector.tensor_tensor(dst, ps[:], b.to_broadcast((P, QT)), ADD)
            else:
                nc.gpsimd.tensor_tensor(dst, ps[:], b.to_broadcast((P, QT)), ADD)

        def copy_v(ps, sc):
            bal.charge("dve", VW, "tt")
            nc.vector.tensor_tensor(v_s[sc][:], ps[:, :VW], bv_sb[:], ADD)

        exp_t = {"act": 0.0, "dve": 0.0}

        def emit_exp_ap(dst, src, units, j):
            # near-strict act/dve alternation (weighted by per-engine exp
            # cost): consecutive in-flight exps must be on different engines
            # or the QK->exp->PV chain serializes
            if j <= 1:
                eng = "act"
            else:
                eng = "act" if exp_t["act"] <= exp_t["dve"] else "dve"
                exp_t[eng] += (units + 222) * 0.833 if eng == "act" \
                    else (units + 120) * 1.04
            bal.charge(eng, units, "ts")
            if eng == "act":
                nc.scalar.activation(dst, src, Exp, scale=0.0625)
            else:
                nc.vector.tensor_scalar(dst.bitcast(U16), src,
                                        BIT_SCALE, BIT_BIAS, MUL, ADD)

        def emit_tp(j, h, on):
            """Transpose o [128q, 64d] -> o^T and store into ot01/ot2.
            Four qc strips go into one psum bank: only the first transpose
            uses start=True (pending-zero write-through covers the rest), so
            nothing serializes against the copy."""
            qsl = slice(QT * j, QT * (j + 1))
            tp = ps_tp.tile([P, 4, P], BF, tag="tp", name=f"tp{j}_{h}")
            rows = slice(D * (h % 2), D * (h % 2) + D)
            for qc in range(4):
                nc.tensor.matmul(tp[rows, qc, :], on[:, qc, :], id_sb[:],
                                 start=(qc == 0), stop=(qc == 3),
                                 is_transpose=True)
            dst = (ot01[rows, qsl] if h < 2 else ot2[:, qsl])
            eng = bal.pick(QT)
            eng_copy(eng, dst.rearrange("p (c q) -> p c q", c=4), tp[rows, :, :])

        osb_live = {}

        def emit_proj_group(jp, i, pool=None):
            """One of 8 projection psum groups for row-tile jp (qc x nh)."""
            qc = 4 * jp + i // 2
            nh = i % 2
            if nh == 0:
                osb_live[qc] = sb_out.tile([P, C], F32, tag="osb",
                                           name=f"osb{qc}")
            osb = osb_live[qc]
            if pool is None:
                pj = ps_pj.tile([P, 384], F32, tag="pj", name=f"pj{qc}_{nh}")
            else:
                pj = pool.tile([P, 384], F32, tag="gen", name=f"pj{qc}_{nh}")
            nsl = slice(384 * nh, 384 * (nh + 1))
            nc.tensor.matmul(pj[:], ot01[:, P * qc:P * (qc + 1)],
                             wo01_sb[:, nsl], start=True, stop=False)
            nc.tensor.matmul(pj[:], ot2[:, P * qc:P * (qc + 1)],
                             wo2_sb[:, nsl], start=False, stop=True)
            eng = bal.pick(384)
            eng_copy(eng, osb[:, nsl], pj[:])
            if nh == 1:
                nc.sync.dma_start(out[P * qc:P * (qc + 1), :], osb[:])
                del osb_live[qc]

        gen_live = {}

        def emit_gen_half(s, i, part, pool=None):
            """Half of a gen psum group (3 of 6 contraction chunks); the
            group stays open across the two halves so filler can interleave
            at sub-group granularity."""
            qsl = slice(QT * s, QT * (s + 1))
            if part == 0:
                pool = pool if pool is not None else ps_gen
                tg = "gen" if pool is ps_gen else "pj"
                gen_live[(s, i)] = pool.tile([P, QT], F32, tag=tg,
                                             name=f"psg{s}_{i}")
            ps = gen_live[(s, i)]
            crange = range(0, 3) if part == 0 else range(3, CC)
            if i < 3:
                g = i
                for c in crange:
                    nc.tensor.matmul(ps[:], wqk_sb[:, c, P * g:P * (g + 1)],
                                     xt_sb[c][:, qsl],
                                     start=(c == 0), stop=(c == CC - 1))
                if part == 1:
                    copy_qk(ps, g, qsl)
                    del gen_live[(s, i)]
                    if g == 2:
                        # replicate q2 to partitions 64-127 so head2's QK
                        # operands share base 64 (DMA engines are idle)
                        nc.sync.dma_start(q2b[D:P, qsl], qk8[2][0:D, qsl])
            else:
                sc = 4 * s + (i - 3)
                for c in crange:
                    nc.tensor.matmul(ps[:, :VW], xt_sb[c][:, P * sc:P * (sc + 1)],
                                     wv_sb[:, c, :],
                                     start=(c == 0), stop=(c == CC - 1))
                if part == 1:
                    copy_v(ps, sc)
                    del gen_live[(s, i)]

        def emit_gen_group(s, i, pool=None):
            emit_gen_half(s, i, 0, pool)
            emit_gen_half(s, i, 1, pool)

        tp_pending = []   # deferred (j, h, o_norm) so transposes don't stall
                          # the PE right behind their norm-mul

        # prologue: only what attention(0) heads 0/1 need up front; the
        # [q2|k2] group rides the first filler ticks so the engines start
        # exp work ~3us earlier
        for i in [0, 1, 3, 4, 5, 6]:
            emit_gen_group(0, i, pool=(ps_pj if i % 2 else None))

        for s in range(NQ):
            # Filler PE work sprinkled between kc chunks: next tile's gen and
            # the previous tile's projection. Gen items first (no deps), proj
            # items only emit once this tile's first tp flush has run (they
            # read ot written by the deferred transposes).
            filler = ([("gen", 0, 2, p) for p in range(2)] if s == 0 else []) \
                + ([("gen", s + 1, i, p) for i in range(7) for p in range(2)]
                   if s + 1 < NQ else []) \
                + ([("proj", s - 1, i, 0) for i in range(8)] if s > 0 else [])
            total_ticks = 3 * 2 * (s + 1)
            stride = max(1, total_ticks // (len(filler) + 1)) if filler else 10**9
            fill_state = {"tick": 0, "idx": 0, "proj_ok": False}

            def emit_filler_item(item):
                kind, a, b, p = item
                if kind == "gen":
                    emit_gen_half(a, b, p)
                else:
                    emit_proj_group(a, b)

            def filler_tick():
                fill_state["tick"] += 1
                while (fill_state["idx"] < len(filler)
                       and fill_state["tick"] >= stride * (fill_state["idx"] + 1)):
                    item = filler[fill_state["idx"]]
                    if item[0] == "proj" and not fill_state["proj_ok"]:
                        return
                    fill_state["idx"] += 1
                    emit_filler_item(item)

            def filler_drain():
                while fill_state["idx"] < len(filler):
                    item = filler[fill_state["idx"]]
                    fill_state["idx"] += 1
                    emit_filler_item(item)

            # ---- attention for q-tile j = s ----
            j = s
            nkc = 4 * (j + 1)
            for h in range(HL):
                if h < 2:
                    qt_, kt_, base = qk8[0], qk8[1], D * h
                else:
                    qt_, kt_, base = q2b, qk8[2], D
                rows = slice(base, base + D)
                # pv bank: qc=0 at kc=0 starts (lazily zeroes the region);
                # the other qc strips write-through via pending-zero.
                pv = ps_pv.tile([P, 4, P], F32, tag="pv", name=f"pv{j}_{h}")

                def emit_pv(pt1, kc, off):
                    for qc in range(off // P, 4):
                        nc.tensor.matmul(pv[:, qc, 0:65],
                                         pt1[:, P * qc:P * (qc + 1)],
                                         v_s[kc][:, 66 * h:66 * h + 65],
                                         start=(kc == 0 and qc == 0),
                                         stop=(kc == nkc - 1 and qc == 3),
                                         skip_group_check=True)

                pv_lag = []
                for kc in range(nkc):
                    diag = kc >= 4 * j
                    off = P * (kc - 4 * j) if diag else 0
                    st1 = ps_st.tile([P, QT], F32, tag="st",
                                     name=f"st{j}_{h}_{kc}")
                    pt1 = sb_pt.tile([P, QT], BF, tag="pt",
                                     name=f"pt{j}_{h}_{kc}")
                    lhsT = kt_[rows, None, P * kc:P * (kc + 1)].to_broadcast(
                        (D, 2, P))
                    rhs = qt_[rows, None,
                              QT * j + off:QT * (j + 1)].to_broadcast(
                        (D, 2, QT - off))
                    nc.tensor.matmul(st1[:, off:], lhsT, rhs,
                                     start=True, stop=not diag,
                                     perf_mode=DR)
                    if diag:
                        nc.tensor.matmul(st1[:, off:off + P],
                                         id_sb[:], mask_sb[:],
                                         start=False, stop=True,
                                         skip_group_check=True)
                    emit_exp_ap(pt1[:, off:], st1[:, off:], QT - off, j)
                    pv_lag.append((pt1, kc, off))
                    if len(pv_lag) > 5:
                        emit_pv(*pv_lag.pop(0))
                    if kc % 2 == 1:
                        filler_tick()
                while pv_lag:
                    emit_pv(*pv_lag.pop(0))
                # normalize: reciprocal of the per-q denominators (col 64)
                dn = sb_dn.tile([P, 4], F32, tag="dn", name=f"dn{j}_{h}")
                nc.vector.reciprocal(dn[:], pv[:, :, 64])
                bal.charge("dve", 4, "tt")
                on = sb_on.tile([P, 4, D], BF, tag="on", name=f"on{j}_{h}")
                nc.vector.tensor_tensor(on[:], pv[:, :, 0:D],
                                        dn[:, :, None].to_broadcast((P, 4, D)),
                                        MUL)
                bal.charge("dve", 4 * D, "tt")
                tp_pending.append((j, h, on))
                while len(tp_pending) > 1:
                    emit_tp(*tp_pending.pop(0))
                if h == 0:
                    fill_state["proj_ok"] = True
            filler_drain()

        # tail: flush remaining transposes, then the last row's projection
        # (alternating psum banks: gen's slots are free by now)
        while tp_pending:
            emit_tp(*tp_pending.pop(0))
        for i in range(8):
            emit_proj_group(NQ - 1, i, pool=(ps_gen if i % 2 == 0 else None))

    return nc


def prep_core_inputs(x, W_attn, b_attn, W_out, b, g):
    """Host-side shard prep for core (batch b, head group g)."""
    habs = [HL * g + h for h in range(HL)]
    wq = [W_attn[:, D * h:D * (h + 1)] for h in habs]
    wk = [W_attn[:, C + D * h:C + D * (h + 1)] for h in habs]
    wv_ = [W_attn[:, 2 * C + D * h:2 * C + D * (h + 1)] for h in habs]
    bq = [b_attn[D * h:D * (h + 1)] for h in habs]
    bk = [b_attn[C + D * h:C + D * (h + 1)] for h in habs]
    bvv = [b_attn[2 * C + D * h:2 * C + D * (h + 1)] for h in habs]

    # [q0|q1] [k0|k1] [q2|k2]; k-bias dropped entirely (it adds a per-row
    # constant to the scores, which softmax shift-invariance cancels)
    wqk = np.concatenate(
        [wq[0], wq[1], wk[0], wk[1], wq[2], wk[2]], axis=1)
    zz = np.zeros(D, dtype=np.float32)
    bqk = np.stack([
        np.concatenate([bq[0], bq[1]]),
        np.concatenate([zz, zz]),
        np.concatenate([bq[2], zz]),
    ], axis=1).astype(np.float32)

    wv_ext = np.zeros((C, VW), dtype=np.float32)
    bv_ext = np.zeros(VW, dtype=np.float32)
    for h in range(HL):
        wv_ext[:, 66 * h:66 * h + D] = wv_[h]
        bv_ext[66 * h:66 * h + D] = bvv[h]
        bv_ext[66 * h + D] = 1.0
    bv_tile = np.ascontiguousarray(
        np.broadcast_to(bv_ext, (P, VW))).astype(np.float32)

    wo_g = W_out[192 * g:192 * (g + 1), :]

    p = np.arange(P)
    # additive causal mask for the diagonal block: -600 where masked (l < p)
    mask = ((p[None, :] < p[:, None]) * -600.0).astype(BF_NP)

    return {
        "xT": np.ascontiguousarray(x[b].T).astype(BF_NP),
        "wqk": wqk.astype(BF_NP),
        "bqk": bqk,
        "wv": wv_ext.astype(BF_NP),
        "bv": bv_tile,
        "wo01": np.ascontiguousarray(wo_g[:P, :]).astype(BF_NP),
        "wo2": np.ascontiguousarray(wo_g[P:, :]).astype(BF_NP),
        "mask": mask,
        "ident": np.eye(P, dtype=BF_NP),
    }


_NC_CACHE = {}


def kernel(x, W_attn, b_attn, W_out, b_out):
    x = np.asarray(x, dtype=np.float32)
    W_attn = np.asarray(W_attn, dtype=np.float32)
    b_attn = np.asarray(b_attn, dtype=np.float32)
    W_out = np.asarray(W_out, dtype=np.float32)
    b_out = np.asarray(b_out, dtype=np.float32)
    B, n_seq, _ = x.shape

    if n_seq not in _NC_CACHE:
        _NC_CACHE[n_seq] = build_nc(n_seq)
    nc = _NC_CACHE[n_seq]

    in_maps = [prep_core_inputs(x, W_attn, b_attn, W_out, b, g)
               for b in range(B) for g in range(4)]
    res = bass_utils.run_bass_kernel_spmd(
        nc, in_maps, core_ids=list(range(N_CORES)))
    parts = [r["out"] for r in res.results]
    out = np.empty((B, n_seq, C), dtype=np.float32)
    for b in range(B):
        out[b] = parts[4 * b] + parts[4 * b + 1] + parts[4 * b + 2] \
            + parts[4 * b + 3] + b_out
    return out


# revision 60
# speedup vs baseline: 1.5515x; 1.0067x over previous
"""Causal self-attention (B=2, N=4096, C=768, 12 heads, d=64) on 8 trn2 cores.

Sharding: core (b, g) = batch b, head-group g (3 heads). Each core computes
the qkv projection for its 3 heads, causal attention, and a partial output
projection; host sums the 4 partials per batch and adds b_out.

Schedule (per core), designed against the TimelineSim cost model where a
matmul costs out_free_size x cycles_per_row (0.5 for fp8 DoubleRow, no
stationary-load cost) and engine ops cost free_size x cycle + fixed init
(gpsimd cannot access PSUM on this target, so only Act/DVE carry the
elementwise work; PE filler keeps the tensor engine busy while they chew):
 - qkv gen: bf16 matmuls, stationary W [128c,128], moving xT [128c, 512].
   Weight-column packing [q0|q1][k0|k1][q2|k2]; q2 is replicated to
   partitions 64-127 by an SBUF->SBUF DMA so head2's QK operands share
   partition base 64. K-bias dropped (softmax shift-invariance).
 - q,k stored fp8e4; QK^T via fp8 DoubleRow with BOTH k-tile slabs aliased
   to the same data (stride-0 slab dim) -> scores come out exactly 2x,
   folded into the exp scale. 0.5 cycles/row halves QK cost vs bf16.
 - causal mask folded into the PE: the diagonal block accumulates
   I^T @ (-600 * masked) inside the score psum group; exp of -1200-ish
   underflows to exactly zero on both exp paths.
 - exp alternates Act (native Exp; j==0 pinned for accuracy since short
   rows average away less noise) and DVE (Schraudolph bf16 bit trick:
   u16 = round(s2*0.0625*log2e*128 + 16198.4), bitcast bf16; ~3% rms
   multiplicative noise, washed out by softmax averaging on long rows).
   Four single-bank score tiles keep 4 exps in flight; PV trails QK by 5
   chunks so the in-order PE never waits on a fresh exp.
 - PV in [q, d] form: stationary probs [128kpos, 128q], moving V_ext
   [128, 65] ([v | ones] -> denominator lands in column 64). One psum bank
   holds all 4 qc accumulation strips: only the first matmul uses
   start=True; pending-zero write-through covers the rest (a zero-region
   is a whole bank, so four proper groups can't share one).
 - normalize per qc chunk with reciprocal of column 64 (per-partition
   scalar -- no cross-partition denominator bounce needed in this layout),
   then PE-transpose o [128q, 64d] -> o^T [64, 128] for the projection.
 - out projection bf16: lhsT = o^T chunks (K=128 packed h0h1 + K=64 h2),
   moving W_out [., 768]; partial written f32 and summed on host.
 - gen(s+1) and proj(s-1) are emitted as fine-grained filler between kc
   chunks, so psum-bank copy latencies hide behind unrelated matmuls.
"""

import numpy as np
import ml_dtypes

import concourse.bass as bass
import concourse.mybir as mybir
import concourse.tile as tile
from concourse import bass_utils
from concourse.vector_clock import ScopedClock

P = 128
D = 64
C = 768
HL = 3          # heads per core
QT = 512        # q tile width
VW = 66 * HL    # v sbuf row width: [v_h(64) | ones | pad] x 3
N_CORES = 8
BF = mybir.dt.bfloat16
F32 = mybir.dt.float32
FP8 = mybir.dt.float8e4
U16 = mybir.dt.uint16
BF_NP = ml_dtypes.bfloat16
FP8_NP = ml_dtypes.float8_e4m3

LOG2E = 1.4426950408889634
# scores arrive 2x-scaled (aliased DoubleRow slabs); exp(s2*0.0625) as
# 2^(s2*0.0625*log2e) via bf16 bit trick with magic bias 16256 - 0.45*128
BIT_SCALE = 0.0625 * LOG2E * 128.0
BIT_BIAS = 16256.0 - 57.6
DR = mybir.MatmulPerfMode.DoubleRow
Exp = mybir.ActivationFunctionType.Exp
Identity = mybir.ActivationFunctionType.Identity
Copy = mybir.ActivationFunctionType.Copy
MUL = mybir.AluOpType.mult
ADD = mybir.AluOpType.add


class PatchedTileContext(tile.TileContext):
    """This toolchain's walrus rejects more than ONE sync-wait on any
    instruction ("Too many sync wait commands"). Tile's wait assignment
    freely attaches several. Legalize: for every instruction with k>1
    waits, insert k-1 same-engine NOPs before it, one wait each."""

    def _split_sync_waits(self):
        nc = self.nc
        for bb in nc.m.functions[0].blocks:
            insts = bb.instructions
            out = []
            changed = False
            for inst in insts:
                si = inst.sync_info
                waits = list(si.on_wait or []) if si is not None else []
                if len(waits) > 1:
                    changed = True
                    for w in waits[:-1]:
                        nop = mybir.InstNoOp(
                            name=f"I-wsplit{nc.next_id()}", text_hint="wsplit")
                        nop.engine = inst.engine
                        nop.sync_info = mybir.SyncInfo(on_wait=[w], on_update=[])
                        nc.register_instruction(nop)
                        out.append(nop)
                    si.on_wait = waits[-1:]
                out.append(inst)
            if changed:
                bb.instructions = out

    def _drain_and_barrier(self, tick_clock, wait_clock):
        drain_inst = self.nc.sync.drain()
        wait_clock.add_sem_waits(
            drain_inst.ins, ScopedClock({None: tick_clock.global_clock})
        )
        si = drain_inst.ins.sync_info
        waits = list(si.on_wait or []) if si is not None else []
        if len(waits) > 1:
            si.on_wait = waits[:1]
            for w in waits[1:]:
                extra = self.nc.sync.drain()
                esi = extra.ins.sync_info
                if esi is None:
                    extra.ins.sync_info = mybir.SyncInfo(on_wait=[w], on_update=[])
                else:
                    esi.on_wait = [w]

        self.nc.all_engine_barrier()
        assert self.sems is not None
        popped = self.nc._tile_sem_poison_stack.pop()
        assert popped is self._sem_poison
        # clear_and_free_semaphores would emit EVENT_SEMAPHORE_RANGE_CLEAR
        # (an InstISA), which this walrus rejects ("ISA wrong length") — and
        # per-sem sem_clear lowers to the same opcode. Skip the clears: this
        # is the only TileContext in the NEFF and NRT re-initializes
        # semaphores per execution (verified empirically by repeated runs).
        self.nc.all_engine_barrier()
        self._split_sync_waits()


class Balancer:
    """Greedy engine picker by modeled cumulative busy-ns."""

    POOL_EFF = {"memset": 1.0, "tt": 0.42, "ts": 0.6, "copy": 0.6}

    def __init__(self):
        self.busy = {"act": 0.0, "dve": 0.0, "pool": 0.0}

    @classmethod
    def cost(cls, eng, units, kind="copy"):
        if eng == "act":
            return (units + 222) * 0.833
        if eng == "dve":
            return (units + 120) * 1.04
        return units * 0.833 / cls.POOL_EFF[kind] + 120  # pool

    def pick(self, units, kind="copy", allowed=("act", "dve")):
        # NB: gpsimd (pool) cannot access PSUM on this target, and every
        # elementwise op here reads PSUM — so only act/dve are usable.
        eng = min(allowed, key=lambda e: self.busy[e] + self.cost(e, units, kind))
        self.busy[eng] += self.cost(eng, units, kind)
        return eng

    def charge(self, eng, units, kind="copy"):
        self.busy[eng] += self.cost(eng, units, kind)


def build_nc(n_seq=4096):
    CC = C // P                  # 6 contraction chunks
    NQ = n_seq // QT             # q tiles
    nc = bass.Bass("TRN2", target_bir_lowering=False, debug=False,
                   num_devices=N_CORES)

    xT = nc.dram_tensor("xT", [C, n_seq], BF, kind="ExternalInput").ap()
    wqk = nc.dram_tensor("wqk", [C, 3 * P], BF, kind="ExternalInput").ap()
    bqk = nc.dram_tensor("bqk", [P, 3], F32, kind="ExternalInput").ap()
    wv = nc.dram_tensor("wv", [C, VW], BF, kind="ExternalInput").ap()
    bv = nc.dram_tensor("bv", [P, VW], F32, kind="ExternalInput").ap()
    wo01 = nc.dram_tensor("wo01", [P, C], BF, kind="ExternalInput").ap()
    wo2 = nc.dram_tensor("wo2", [D, C], BF, kind="ExternalInput").ap()
    mask = nc.dram_tensor("mask", [P, P], BF, kind="ExternalInput").ap()
    ident = nc.dram_tensor("ident", [P, P], BF, kind="ExternalInput").ap()
    out = nc.dram_tensor("out", [n_seq, C], F32, kind="ExternalOutput").ap()

    bal = Balancer()

    from contextlib import ExitStack
    with PatchedTileContext(nc) as tc, ExitStack() as ctx:
        consts = ctx.enter_context(tc.tile_pool(name="consts", bufs=1))
        # DMA order: what the first gen group needs comes first (xT row-tile
        # 0 pieces + first wqk column group), then the rest of the weights,
        # then the remaining xT pieces s-major.
        xt_sb = [consts.tile([P, n_seq], BF, name=f"xt{c}") for c in range(CC)]
        wqk_sb = consts.tile([P, CC, 3 * P], BF, name="wqk_sb")
        wqk_r = wqk.rearrange("(o p) m -> p o m", p=P)
        for c in range(CC):
            nc.sync.dma_start(xt_sb[c][:, 0:QT], xT[c * P:(c + 1) * P, 0:QT])
        for g in range(3):
            nc.sync.dma_start(wqk_sb[:, :, P * g:P * (g + 1)],
                              wqk_r[:, :, P * g:P * (g + 1)])
        bqk_sb = consts.tile([P, 3], F32, name="bqk_sb")
        nc.sync.dma_start(bqk_sb[:], bqk[:])
        wv_sb = consts.tile([P, CC, VW], BF, name="wv_sb")
        nc.sync.dma_start(wv_sb[:], wv.rearrange("(o p) m -> p o m", p=P))
        bv_sb = consts.tile([P, VW], F32, name="bv_sb")
        nc.sync.dma_start(bv_sb[:], bv[:])
        wo01_sb = consts.tile([P, C], BF, name="wo01_sb")
        nc.sync.dma_start(wo01_sb[:], wo01[:])
        wo2_sb = consts.tile([D, C], BF, name="wo2_sb")
        nc.sync.dma_start(wo2_sb[:], wo2[:])
        mask_sb = consts.tile([P, P], BF, name="mask_sb")
        nc.sync.dma_start(mask_sb[:], mask[:])
        id_sb = consts.tile([P, P], BF, name="id_sb")
        nc.sync.dma_start(id_sb[:], ident[:])
        for s in range(1, NQ):
            for c in range(CC):
                nc.sync.dma_start(xt_sb[c][:, QT * s:QT * (s + 1)],
                                  xT[c * P:(c + 1) * P, QT * s:QT * (s + 1)])
        # persistent intermediates: [q0|q1], [k0|k1], [q2|k2], and a dup
        # tile whose bottom half receives q2 via SBUF->SBUF DMA so head2's
        # QK operands share partition base 64
        qk8 = [consts.tile([P, n_seq], FP8, name=f"qk8_{g}") for g in range(3)]
        q2b = consts.tile([P, n_seq], FP8, name="q2b")
        v_s = [consts.tile([P, VW], BF, name=f"v{sc}") for sc in range(4 * NQ)]
        ot01 = consts.tile([P, n_seq], BF, name="ot01")
        ot2 = consts.tile([D, n_seq], BF, name="ot2")

        ps_gen = ctx.enter_context(tc.tile_pool(name="ps_gen", bufs=1, space="PSUM"))
        ps_st = ctx.enter_context(tc.tile_pool(name="ps_st", bufs=4, space="PSUM"))
        ps_pv = ctx.enter_context(tc.tile_pool(name="ps_pv", bufs=1, space="PSUM"))
        ps_tp = ctx.enter_context(tc.tile_pool(name="ps_tp", bufs=1, space="PSUM"))
        ps_pj = ctx.enter_context(tc.tile_pool(name="ps_pj", bufs=1, space="PSUM"))
        sb_pt = ctx.enter_context(tc.tile_pool(name="sb_pt", bufs=9))
        sb_on = ctx.enter_context(tc.tile_pool(name="sb_on", bufs=4))
        sb_dn = ctx.enter_context(tc.tile_pool(name="sb_dn", bufs=3))
        sb_out = ctx.enter_context(tc.tile_pool(name="sb_out", bufs=3))

        def eng_copy(eng, dst, src):
            if eng == "act":
                nc.scalar.activation(dst, src, Copy)
            elif eng == "dve":
                nc.vector.tensor_copy(dst, src)
            else:
                nc.gpsimd.tensor_copy(dst, src)

        def copy_qk(ps, g, qsl):
            """PSUM f32 + per-partition bias -> fp8 q/k tile."""
            eng = bal.pick(QT, "tt")
            dst = qk8[g][:, qsl]
            b = bqk_sb[:, g:g + 1]
            if eng == "act":
                nc.scalar.activation(dst, ps[:], Identity, bias=b, scale=1.0)
            elif eng == "dve":
                nc.vector.tensor_tensor(dst, ps[:], b.to_broadcast((P, QT)), ADD)
            else:
                nc.gpsimd.tensor_tensor(dst, ps[:], b.to_broadcast((P, QT)), ADD)

        def copy_v(ps, sc):
            bal.charge("dve", VW, "tt")
            nc.vector.tensor_tensor(v_s[sc][:], ps[:, :VW], bv_sb[:], ADD)

        exp_t = {"act": 0.0, "dve": 0.0}

        def emit_exp_ap(dst, src, units, j):
            # near-strict act/dve alternation (weighted by per-engine exp
            # cost): consecutive in-flight exps must be on different engines
            # or the QK->exp->PV chain serializes
            if j == 0:
                eng = "act"
            else:
                eng = "act" if exp_t["act"] <= exp_t["dve"] else "dve"
                exp_t[eng] += (units + 222) * 0.833 if eng == "act" \
                    else (units + 120) * 1.04
            bal.charge(eng, units, "ts")
            if eng == "act":
                nc.scalar.activation(dst, src, Exp, scale=0.0625)
            else:
                nc.vector.tensor_scalar(dst.bitcast(U16), src,
                                        BIT_SCALE, BIT_BIAS, MUL, ADD)

        def emit_tp(j, h, on):
            """Transpose o [128q, 64d] -> o^T and store into ot01/ot2.
            Four qc strips go into one psum bank: only the first transpose
            uses start=True (pending-zero write-through covers the rest), so
            nothing serializes against the copy."""
            qsl = slice(QT * j, QT * (j + 1))
            tp = ps_tp.tile([P, 4, P], BF, tag="tp", name=f"tp{j}_{h}")
            rows = slice(D * (h % 2), D * (h % 2) + D)
            for qc in range(4):
                nc.tensor.matmul(tp[rows, qc, :], on[:, qc, :], id_sb[:],
                                 start=(qc == 0), stop=(qc == 3),
                                 is_transpose=True)
            dst = (ot01[rows, qsl] if h < 2 else ot2[:, qsl])
            eng = bal.pick(QT)
            eng_copy(eng, dst.rearrange("p (c q) -> p c q", c=4), tp[rows, :, :])

        osb_live = {}

        def emit_proj_group(jp, i, pool=None):
            """One of 8 projection psum groups for row-tile jp (qc x nh)."""
            qc = 4 * jp + i // 2
            nh = i % 2
            if nh == 0:
                osb_live[qc] = sb_out.tile([P, C], F32, tag="osb",
                                           name=f"osb{qc}")
            osb = osb_live[qc]
            if pool is None:
                pj = ps_pj.tile([P, 384], F32, tag="pj", name=f"pj{qc}_{nh}")
            else:
                pj = pool.tile([P, 384], F32, tag="gen", name=f"pj{qc}_{nh}")
            nsl = slice(384 * nh, 384 * (nh + 1))
            nc.tensor.matmul(pj[:], ot01[:, P * qc:P * (qc + 1)],
                             wo01_sb[:, nsl], start=True, stop=False)
            nc.tensor.matmul(pj[:], ot2[:, P * qc:P * (qc + 1)],
                             wo2_sb[:, nsl], start=False, stop=True)
            eng = bal.pick(384)
            eng_copy(eng, osb[:, nsl], pj[:])
            if nh == 1:
                nc.sync.dma_start(out[P * qc:P * (qc + 1), :], osb[:])
                del osb_live[qc]

        gen_live = {}

        def emit_gen_half(s, i, part, pool=None):
            """Half of a gen psum group (3 of 6 contraction chunks); the
            group stays open across the two halves so filler can interleave
            at sub-group granularity."""
            qsl = slice(QT * s, QT * (s + 1))
            if part == 0:
                pool = pool if pool is not None else ps_gen
                tg = "gen" if pool is ps_gen else "pj"
                gen_live[(s, i)] = pool.tile([P, QT], F32, tag=tg,
                                             name=f"psg{s}_{i}")
            ps = gen_live[(s, i)]
            crange = range(0, 3) if part == 0 else range(3, CC)
            if i < 3:
                g = i
                for c in crange:
                    nc.tensor.matmul(ps[:], wqk_sb[:, c, P * g:P * (g + 1)],
                                     xt_sb[c][:, qsl],
                                     start=(c == 0), stop=(c == CC - 1))
                if part == 1:
                    copy_qk(ps, g, qsl)
                    del gen_live[(s, i)]
                    if g == 2:
                        # replicate q2 to partitions 64-127 so head2's QK
                        # operands share base 64 (DMA engines are idle)
                        nc.sync.dma_start(q2b[D:P, qsl], qk8[2][0:D, qsl])
            else:
                sc = 4 * s + (i - 3)
                for c in crange:
                    nc.tensor.matmul(ps[:, :VW], xt_sb[c][:, P * sc:P * (sc + 1)],
                                     wv_sb[:, c, :],
                                     start=(c == 0), stop=(c == CC - 1))
                if part == 1:
                    copy_v(ps, sc)
                    del gen_live[(s, i)]

        def emit_gen_group(s, i, pool=None):
            emit_gen_half(s, i, 0, pool)
            emit_gen_half(s, i, 1, pool)

        tp_pending = []   # deferred (j, h, o_norm) so transposes don't stall
                          # the PE right behind their norm-mul

        # prologue: only what attention(0) heads 0/1 need up front; the
        # [q2|k2] group rides the first filler ticks so the engines start
        # exp work ~3us earlier
        for i in [0, 1, 3, 4, 5, 6]:
            emit_gen_group(0, i, pool=(ps_pj if i % 2 else None))

        for s in range(NQ):
            # Filler PE work sprinkled between kc chunks: next tile's gen and
            # the previous tile's projection. Gen items first (no deps), proj
            # items only emit once this tile's first tp flush has run (they
            # read ot written by the deferred transposes).
            filler = ([("gen", 0, 2, p) for p in range(2)] if s == 0 else []) \
                + ([("gen", s + 1, i, p) for i in range(7) for p in range(2)]
                   if s + 1 < NQ else []) \
                + ([("proj", s - 1, i, 0) for i in range(8)] if s > 0 else [])
            total_ticks = 3 * 2 * (s + 1)
            stride = max(1, total_ticks // (len(filler) + 1)) if filler else 10**9
            fill_state = {"tick": 0, "idx": 0, "proj_ok": False}

            def emit_filler_item(item):
                kind, a, b, p = item
                if kind == "gen":
                    emit_gen_half(a, b, p)
                else:
                    emit_proj_group(a, b)

            def filler_tick():
                fill_state["tick"] += 1
                while (fill_state["idx"] < len(filler)
                       and fill_state["tick"] >= stride * (fill_state["idx"] + 1)):
                    item = filler[fill_state["idx"]]
                    if item[0] == "proj" and not fill_state["proj_ok"]:
                        return
                    fill_state["idx"] += 1
                    emit_filler_item(item)

            def filler_drain():
                while fill_state["idx"] < len(filler):
                    item = filler[fill_state["idx"]]
                    fill_state["idx"] += 1
                    emit_filler_item(item)

            # ---- attention for q-tile j = s ----
            j = s
            nkc = 4 * (j + 1)
            for h in range(HL):
                if h < 2:
                    qt_, kt_, base = qk8[0], qk8[1], D * h
                else:
                    qt_, kt_, base = q2b, qk8[2], D
                rows = slice(base, base + D)
                # pv bank: qc=0 at kc=0 starts (lazily zeroes the region);
                # the other qc strips write-through via pending-zero.
                pv = ps_pv.tile([P, 4, P], F32, tag="pv", name=f"pv{j}_{h}")

                def emit_pv(pt1, kc, off):
                    for qc in range(off // P, 4):
                        nc.tensor.matmul(pv[:, qc, 0:65],
                                         pt1[:, P * qc:P * (qc + 1)],
                                         v_s[kc][:, 66 * h:66 * h + 65],
                                         start=(kc == 0 and qc == 0),
                                         stop=(kc == nkc - 1 and qc == 3),
                                         skip_group_check=True)

                pv_lag = []
                for kc in range(nkc):
                    diag = kc >= 4 * j
                    off = P * (kc - 4 * j) if diag else 0
                    st1 = ps_st.tile([P, QT], F32, tag="st",
                                     name=f"st{j}_{h}_{kc}")
                    pt1 = sb_pt.tile([P, QT], BF, tag="pt",
                                     name=f"pt{j}_{h}_{kc}")
                    lhsT = kt_[rows, None, P * kc:P * (kc + 1)].to_broadcast(
                        (D, 2, P))
                    rhs = qt_[rows, None,
                              QT * j + off:QT * (j + 1)].to_broadcast(
                        (D, 2, QT - off))
                    nc.tensor.matmul(st1[:, off:], lhsT, rhs,
                                     start=True, stop=not diag,
                                     perf_mode=DR)
                    if diag:
                        nc.tensor.matmul(st1[:, off:off + P],
                                         id_sb[:], mask_sb[:],
                                         start=False, stop=True,
                                         skip_group_check=True)
                    emit_exp_ap(pt1[:, off:], st1[:, off:], QT - off, j)
                    pv_lag.append((pt1, kc, off))
                    if len(pv_lag) > 5:
                        emit_pv(*pv_lag.pop(0))
                    if kc % 2 == 1:
                        filler_tick()
                while pv_lag:
                    emit_pv(*pv_lag.pop(0))
                # normalize: reciprocal of the per-q denominators (col 64)
                dn = sb_dn.tile([P, 4], F32, tag="dn", name=f"dn{j}_{h}")
                nc.vector.reciprocal(dn[:], pv[:, :, 64])
                bal.charge("dve", 4, "tt")
                on = sb_on.tile([P, 4, D], BF, tag="on", name=f"on{j}_{h}")
                nc.vector.tensor_tensor(on[:], pv[:, :, 0:D],
                                        dn[:, :, None].to_broadcast((P, 4, D)),
                                        MUL)
                bal.charge("dve", 4 * D, "tt")
                tp_pending.append((j, h, on))
                while len(tp_pending) > 1:
                    emit_tp(*tp_pending.pop(0))
                if h == 0:
                    fill_state["proj_ok"] = True
            filler_drain()

        # tail: flush remaining transposes, then the last row's projection
        # (alternating psum banks: gen's slots are free by now)
        while tp_pending:
            emit_tp(*tp_pending.pop(0))
        for i in range(8):
            emit_proj_group(NQ - 1, i, pool=(ps_gen if i % 2 == 0 else None))

    return nc


def prep_core_inputs(x, W_attn, b_attn, W_out, b, g):
    """Host-side shard prep for core (batch b, head group g)."""
    habs = [HL * g + h for h in range(HL)]
    wq = [W_attn[:, D * h:D * (h + 1)] for h in habs]
    wk = [W_attn[:, C + D * h:C + D * (h + 1)] for h in habs]
    wv_ = [W_attn[:, 2 * C + D * h:2 * C + D * (h + 1)] for h in habs]
    bq = [b_attn[D * h:D * (h + 1)] for h in habs]
    bk = [b_attn[C + D * h:C + D * (h + 1)] for h in habs]
    bvv = [b_attn[2 * C + D * h:2 * C + D * (h + 1)] for h in habs]

    # [q0|q1] [k0|k1] [q2|k2]; k-bias dropped entirely (it adds a per-row
    # constant to the scores, which softmax shift-invariance cancels)
    wqk = np.concatenate(
        [wq[0], wq[1], wk[0], wk[1], wq[2], wk[2]], axis=1)
    zz = np.zeros(D, dtype=np.float32)
    bqk = np.stack([
        np.concatenate([bq[0], bq[1]]),
        np.concatenate([zz, zz]),
        np.concatenate([bq[2], zz]),
    ], axis=1).astype(np.float32)

    wv_ext = np.zeros((C, VW), dtype=np.float32)
    bv_ext = np.zeros(VW, dtype=np.float32)
    for h in range(HL):
        wv_ext[:, 66 * h:66 * h + D] = wv_[h]
        bv_ext[66 * h:66 * h + D] = bvv[h]
        bv_ext[66 * h + D] = 1.0
    bv_tile = np.ascontiguousarray(
        np.broadcast_to(bv_ext, (P, VW))).astype(np.float32)

    wo_g = W_out[192 * g:192 * (g + 1), :]

    p = np.arange(P)
    # additive causal mask for the diagonal block: -600 where masked (l < p)
    mask = ((p[None, :] < p[:, None]) * -600.0).astype(BF_NP)

    return {
        "xT": np.ascontiguousarray(x[b].T).astype(BF_NP),
        "wqk": wqk.astype(BF_NP),
        "bqk": bqk,
        "wv": wv_ext.astype(BF_NP),
        "bv": bv_tile,
        "wo01": np.ascontiguousarray(wo_g[:P, :]).astype(BF_NP),
        "wo2": np.ascontiguousarray(wo_g[P:, :]).astype(BF_NP),
        "mask": mask,
        "ident": np.eye(P, dtype=BF_NP),
    }


_NC_CACHE = {}


def kernel(x, W_attn, b_attn, W_out, b_out):
    x = np.asarray(x, dtype=np.float32)
    W_attn = np.asarray(W_attn, dtype=np.float32)
    b_attn = np.asarray(b_attn, dtype=np.float32)
    W_out = np.asarray(W_out, dtype=np.float32)
    b_out = np.asarray(b_out, dtype=np.float32)
    B, n_seq, _ = x.shape

    if n_seq not in _NC_CACHE:
        _NC_CACHE[n_seq] = build_nc(n_seq)
    nc = _NC_CACHE[n_seq]

    in_maps = [prep_core_inputs(x, W_attn, b_attn, W_out, b, g)
               for b in range(B) for g in range(4)]
    res = bass_utils.run_bass_kernel_spmd(
        nc, in_maps, core_ids=list(range(N_CORES)))
    parts = [r["out"] for r in res.results]
    out = np.empty((B, n_seq, C), dtype=np.float32)
    for b in range(B):
        out[b] = parts[4 * b] + parts[4 * b + 1] + parts[4 * b + 2] \
            + parts[4 * b + 3] + b_out
    return out


# revision 61
# speedup vs baseline: 1.6142x; 1.0405x over previous
"""Causal self-attention (B=2, N=4096, C=768, 12 heads, d=64) on 8 trn2 cores.

Sharding: core (b, g) = batch b, head-group g (3 heads). Each core computes
the qkv projection for its 3 heads, causal attention, and a partial output
projection; host sums the 4 partials per batch and adds b_out.

Schedule (per core), designed against the TimelineSim cost model where a
matmul costs out_free_size x cycles_per_row (0.5 for fp8 DoubleRow, no
stationary-load cost) and engine ops cost free_size x cycle + fixed init
(gpsimd cannot access PSUM on this target, so only Act/DVE carry the
elementwise work; PE filler keeps the tensor engine busy while they chew):
 - qkv gen: bf16 matmuls, stationary W [128c,128], moving xT [128c, 512].
   Weight-column packing [q0|q1][k0|k1][q2|k2]; q2 is replicated to
   partitions 64-127 by an SBUF->SBUF DMA so head2's QK operands share
   partition base 64. K-bias dropped (softmax shift-invariance).
 - q,k stored fp8e4; QK^T via fp8 DoubleRow with BOTH k-tile slabs aliased
   to the same data (stride-0 slab dim) -> scores come out exactly 2x,
   folded into the exp scale. 0.5 cycles/row halves QK cost vs bf16.
 - causal mask folded into the PE: the diagonal block accumulates
   I^T @ (-600 * masked) inside the score psum group; exp of -1200-ish
   underflows to exactly zero on both exp paths.
 - exp alternates Act (native Exp; j==0 pinned for accuracy since short
   rows average away less noise) and DVE (Schraudolph bf16 bit trick:
   u16 = round(s2*0.0625*log2e*128 + 16198.4), bitcast bf16; ~3% rms
   multiplicative noise, washed out by softmax averaging on long rows).
   Four single-bank score tiles keep 4 exps in flight; PV trails QK by 5
   chunks so the in-order PE never waits on a fresh exp.
 - PV in [q, d] form: stationary probs [128kpos, 128q], moving V_ext
   [128, 65] ([v | ones] -> denominator lands in column 64). One psum bank
   holds all 4 qc accumulation strips: only the first matmul uses
   start=True; pending-zero write-through covers the rest (a zero-region
   is a whole bank, so four proper groups can't share one).
 - normalize per qc chunk with reciprocal of column 64 (per-partition
   scalar -- no cross-partition denominator bounce needed in this layout),
   then PE-transpose o [128q, 64d] -> o^T [64, 128] for the projection.
 - out projection bf16: lhsT = o^T chunks (K=128 packed h0h1 + K=64 h2),
   moving W_out [., 768]; partial written f32 and summed on host.
 - gen(s+1) and proj(s-1) are emitted as fine-grained filler between kc
   chunks, so psum-bank copy latencies hide behind unrelated matmuls.
"""

import numpy as np
import ml_dtypes

import concourse.bass as bass
import concourse.mybir as mybir
import concourse.tile as tile
from concourse import bass_utils
from concourse.vector_clock import ScopedClock

P = 128
D = 64
C = 768
HL = 3          # heads per core
QT = 512        # q tile width
VW = 66 * HL    # v sbuf row width: [v_h(64) | ones | pad] x 3
N_CORES = 8
BF = mybir.dt.bfloat16
F32 = mybir.dt.float32
FP8 = mybir.dt.float8e4
U16 = mybir.dt.uint16
BF_NP = ml_dtypes.bfloat16
FP8_NP = ml_dtypes.float8_e4m3

LOG2E = 1.4426950408889634
# scores arrive 2x-scaled (aliased DoubleRow slabs); exp(s2*0.0625) as
# 2^(s2*0.0625*log2e) via bf16 bit trick with magic bias 16256 - 0.45*128
BIT_SCALE = 0.0625 * LOG2E * 128.0
BIT_BIAS = 16256.0 - 57.6
DR = mybir.MatmulPerfMode.DoubleRow
Exp = mybir.ActivationFunctionType.Exp
Identity = mybir.ActivationFunctionType.Identity
Copy = mybir.ActivationFunctionType.Copy
MUL = mybir.AluOpType.mult
ADD = mybir.AluOpType.add


class PatchedTileContext(tile.TileContext):
    """This toolchain's walrus rejects more than ONE sync-wait on any
    instruction ("Too many sync wait commands"). Tile's wait assignment
    freely attaches several. Legalize: for every instruction with k>1
    waits, insert k-1 same-engine NOPs before it, one wait each."""

    def _split_sync_waits(self):
        nc = self.nc
        for bb in nc.m.functions[0].blocks:
            insts = bb.instructions
            out = []
            changed = False
            for inst in insts:
                si = inst.sync_info
                waits = list(si.on_wait or []) if si is not None else []
                if len(waits) > 1:
                    changed = True
                    for w in waits[:-1]:
                        nop = mybir.InstNoOp(
                            name=f"I-wsplit{nc.next_id()}", text_hint="wsplit")
                        nop.engine = inst.engine
                        nop.sync_info = mybir.SyncInfo(on_wait=[w], on_update=[])
                        nc.register_instruction(nop)
                        out.append(nop)
                    si.on_wait = waits[-1:]
                out.append(inst)
            if changed:
                bb.instructions = out

    def _drain_and_barrier(self, tick_clock, wait_clock):
        drain_inst = self.nc.sync.drain()
        wait_clock.add_sem_waits(
            drain_inst.ins, ScopedClock({None: tick_clock.global_clock})
        )
        si = drain_inst.ins.sync_info
        waits = list(si.on_wait or []) if si is not None else []
        if len(waits) > 1:
            si.on_wait = waits[:1]
            for w in waits[1:]:
                extra = self.nc.sync.drain()
                esi = extra.ins.sync_info
                if esi is None:
                    extra.ins.sync_info = mybir.SyncInfo(on_wait=[w], on_update=[])
                else:
                    esi.on_wait = [w]

        self.nc.all_engine_barrier()
        assert self.sems is not None
        popped = self.nc._tile_sem_poison_stack.pop()
        assert popped is self._sem_poison
        # clear_and_free_semaphores would emit EVENT_SEMAPHORE_RANGE_CLEAR
        # (an InstISA), which this walrus rejects ("ISA wrong length") — and
        # per-sem sem_clear lowers to the same opcode. Skip the clears: this
        # is the only TileContext in the NEFF and NRT re-initializes
        # semaphores per execution (verified empirically by repeated runs).
        self.nc.all_engine_barrier()
        self._split_sync_waits()


class Balancer:
    """Greedy engine picker by modeled cumulative busy-ns."""

    POOL_EFF = {"memset": 1.0, "tt": 0.42, "ts": 0.6, "copy": 0.6}

    def __init__(self):
        self.busy = {"act": 0.0, "dve": 0.0, "pool": 0.0}

    @classmethod
    def cost(cls, eng, units, kind="copy"):
        if eng == "act":
            return (units + 222) * 0.833
        if eng == "dve":
            return (units + 120) * 1.04
        return units * 0.833 / cls.POOL_EFF[kind] + 120  # pool

    def pick(self, units, kind="copy", allowed=("act", "dve")):
        # NB: gpsimd (pool) cannot access PSUM on this target, and every
        # elementwise op here reads PSUM — so only act/dve are usable.
        eng = min(allowed, key=lambda e: self.busy[e] + self.cost(e, units, kind))
        self.busy[eng] += self.cost(eng, units, kind)
        return eng

    def charge(self, eng, units, kind="copy"):
        self.busy[eng] += self.cost(eng, units, kind)


def build_nc(n_seq=4096):
    CC = C // P                  # 6 contraction chunks
    NQ = n_seq // QT             # q tiles
    nc = bass.Bass("TRN2", target_bir_lowering=False, debug=False,
                   num_devices=N_CORES)

    xT = nc.dram_tensor("xT", [C, n_seq], BF, kind="ExternalInput").ap()
    wqk = nc.dram_tensor("wqk", [C, 3 * P], BF, kind="ExternalInput").ap()
    bqk = nc.dram_tensor("bqk", [P, 3], F32, kind="ExternalInput").ap()
    wv = nc.dram_tensor("wv", [C, VW], BF, kind="ExternalInput").ap()
    bv = nc.dram_tensor("bv", [P, VW], F32, kind="ExternalInput").ap()
    wo01 = nc.dram_tensor("wo01", [P, C], BF, kind="ExternalInput").ap()
    wo2 = nc.dram_tensor("wo2", [D, C], BF, kind="ExternalInput").ap()
    mask = nc.dram_tensor("mask", [P, P], BF, kind="ExternalInput").ap()
    ident = nc.dram_tensor("ident", [P, P], BF, kind="ExternalInput").ap()
    out = nc.dram_tensor("out", [n_seq, C], F32, kind="ExternalOutput").ap()

    bal = Balancer()

    from contextlib import ExitStack
    with PatchedTileContext(nc) as tc, ExitStack() as ctx:
        consts = ctx.enter_context(tc.tile_pool(name="consts", bufs=1))
        # DMA order: what the first gen group needs comes first (xT row-tile
        # 0 pieces + first wqk column group), then the rest of the weights,
        # then the remaining xT pieces s-major.
        xt_sb = [consts.tile([P, n_seq], BF, name=f"xt{c}") for c in range(CC)]
        wqk_sb = consts.tile([P, CC, 3 * P], BF, name="wqk_sb")
        wqk_r = wqk.rearrange("(o p) m -> p o m", p=P)
        for c in range(CC):
            nc.sync.dma_start(xt_sb[c][:, 0:QT], xT[c * P:(c + 1) * P, 0:QT])
        for g in range(3):
            nc.sync.dma_start(wqk_sb[:, :, P * g:P * (g + 1)],
                              wqk_r[:, :, P * g:P * (g + 1)])
        bqk_sb = consts.tile([P, 3], F32, name="bqk_sb")
        nc.sync.dma_start(bqk_sb[:], bqk[:])
        wv_sb = consts.tile([P, CC, VW], BF, name="wv_sb")
        nc.sync.dma_start(wv_sb[:], wv.rearrange("(o p) m -> p o m", p=P))
        bv_sb = consts.tile([P, VW], F32, name="bv_sb")
        nc.sync.dma_start(bv_sb[:], bv[:])
        wo01_sb = consts.tile([P, C], BF, name="wo01_sb")
        nc.sync.dma_start(wo01_sb[:], wo01[:])
        wo2_sb = consts.tile([D, C], BF, name="wo2_sb")
        nc.sync.dma_start(wo2_sb[:], wo2[:])
        mask_sb = consts.tile([P, P], BF, name="mask_sb")
        nc.sync.dma_start(mask_sb[:], mask[:])
        id_sb = consts.tile([P, P], BF, name="id_sb")
        nc.sync.dma_start(id_sb[:], ident[:])
        for s in range(1, NQ):
            for c in range(CC):
                nc.sync.dma_start(xt_sb[c][:, QT * s:QT * (s + 1)],
                                  xT[c * P:(c + 1) * P, QT * s:QT * (s + 1)])
        # persistent intermediates: [q0|q1], [k0|k1], [q2|k2], and a dup
        # tile whose bottom half receives q2 via SBUF->SBUF DMA so head2's
        # QK operands share partition base 64
        qk8 = [consts.tile([P, n_seq], FP8, name=f"qk8_{g}") for g in range(3)]
        q2b = consts.tile([P, n_seq], FP8, name="q2b")
        v_s = [consts.tile([P, VW], BF, name=f"v{sc}") for sc in range(4 * NQ)]
        ot01 = consts.tile([P, n_seq], BF, name="ot01")
        ot2 = consts.tile([D, n_seq], BF, name="ot2")

        ps_gen = ctx.enter_context(tc.tile_pool(name="ps_gen", bufs=1, space="PSUM"))
        ps_st = ctx.enter_context(tc.tile_pool(name="ps_st", bufs=4, space="PSUM"))
        ps_pv = ctx.enter_context(tc.tile_pool(name="ps_pv", bufs=1, space="PSUM"))
        ps_tp = ctx.enter_context(tc.tile_pool(name="ps_tp", bufs=1, space="PSUM"))
        ps_pj = ctx.enter_context(tc.tile_pool(name="ps_pj", bufs=1, space="PSUM"))
        sb_pt = ctx.enter_context(tc.tile_pool(name="sb_pt", bufs=9))
        sb_on = ctx.enter_context(tc.tile_pool(name="sb_on", bufs=4))
        sb_dn = ctx.enter_context(tc.tile_pool(name="sb_dn", bufs=3))
        sb_out = ctx.enter_context(tc.tile_pool(name="sb_out", bufs=4))

        def eng_copy(eng, dst, src):
            if eng == "act":
                nc.scalar.activation(dst, src, Copy)
            elif eng == "dve":
                nc.vector.tensor_copy(dst, src)
            else:
                nc.gpsimd.tensor_copy(dst, src)

        def copy_qk(ps, g, qsl):
            """PSUM f32 + per-partition bias -> fp8 q/k tile."""
            eng = bal.pick(QT, "tt")
            dst = qk8[g][:, qsl]
            b = bqk_sb[:, g:g + 1]
            if eng == "act":
                nc.scalar.activation(dst, ps[:], Identity, bias=b, scale=1.0)
            elif eng == "dve":
                nc.vector.tensor_tensor(dst, ps[:], b.to_broadcast((P, QT)), ADD)
            else:
                nc.gpsimd.tensor_tensor(dst, ps[:], b.to_broadcast((P, QT)), ADD)

        def copy_v(ps, sc):
            bal.charge("dve", VW, "tt")
            nc.vector.tensor_tensor(v_s[sc][:], ps[:, :VW], bv_sb[:], ADD)

        exp_t = {"act": 0.0, "dve": 0.0}

        def emit_exp_ap(dst, src, units, j):
            # near-strict act/dve alternation (weighted by per-engine exp
            # cost): consecutive in-flight exps must be on different engines
            # or the QK->exp->PV chain serializes
            if j == 0:
                eng = "act"
            else:
                eng = "act" if exp_t["act"] <= exp_t["dve"] else "dve"
                exp_t[eng] += (units + 222) * 0.833 if eng == "act" \
                    else (units + 120) * 1.04
            bal.charge(eng, units, "ts")
            if eng == "act":
                nc.scalar.activation(dst, src, Exp, scale=0.0625)
            else:
                nc.vector.tensor_scalar(dst.bitcast(U16), src,
                                        BIT_SCALE, BIT_BIAS, MUL, ADD)

        def emit_tp(j, h, on):
            """Transpose o [128q, 64d] -> o^T and store into ot01/ot2.
            Four qc strips go into one psum bank: only the first transpose
            uses start=True (pending-zero write-through covers the rest), so
            nothing serializes against the copy."""
            qsl = slice(QT * j, QT * (j + 1))
            tp = ps_tp.tile([P, 4, P], BF, tag="tp", name=f"tp{j}_{h}")
            rows = slice(D * (h % 2), D * (h % 2) + D)
            for qc in range(4):
                nc.tensor.matmul(tp[rows, qc, :], on[:, qc, :], id_sb[:],
                                 start=(qc == 0), stop=(qc == 3),
                                 is_transpose=True)
            dst = (ot01[rows, qsl] if h < 2 else ot2[:, qsl])
            eng = bal.pick(QT)
            eng_copy(eng, dst.rearrange("p (c q) -> p c q", c=4), tp[rows, :, :])

        osb_live = {}

        def emit_proj_group(jp, i, pool=None):
            """One of 8 projection psum groups for row-tile jp (qc x nh)."""
            qc = 4 * jp + i // 2
            nh = i % 2
            if nh == 0:
                osb_live[qc] = sb_out.tile([P, C], F32, tag="osb",
                                           name=f"osb{qc}")
            osb = osb_live[qc]
            if pool is None:
                pj = ps_pj.tile([P, 384], F32, tag="pj", name=f"pj{qc}_{nh}")
            else:
                pj = pool.tile([P, 384], F32, tag="gen", name=f"pj{qc}_{nh}")
            nsl = slice(384 * nh, 384 * (nh + 1))
            nc.tensor.matmul(pj[:], ot01[:, P * qc:P * (qc + 1)],
                             wo01_sb[:, nsl], start=True, stop=False)
            nc.tensor.matmul(pj[:], ot2[:, P * qc:P * (qc + 1)],
                             wo2_sb[:, nsl], start=False, stop=True)
            eng = bal.pick(384)
            eng_copy(eng, osb[:, nsl], pj[:])
            if nh == 1:
                nc.sync.dma_start(out[P * qc:P * (qc + 1), :], osb[:])
                del osb_live[qc]

        gen_live = {}

        def emit_gen_half(s, i, part, pool=None):
            """Half of a gen psum group (3 of 6 contraction chunks); the
            group stays open across the two halves so filler can interleave
            at sub-group granularity."""
            qsl = slice(QT * s, QT * (s + 1))
            if part == 0:
                pool = pool if pool is not None else ps_gen
                tg = "gen" if pool is ps_gen else "pj"
                gen_live[(s, i)] = pool.tile([P, QT], F32, tag=tg,
                                             name=f"psg{s}_{i}")
            ps = gen_live[(s, i)]
            crange = range(0, 3) if part == 0 else range(3, CC)
            if i < 3:
                g = i
                for c in crange:
                    nc.tensor.matmul(ps[:], wqk_sb[:, c, P * g:P * (g + 1)],
                                     xt_sb[c][:, qsl],
                                     start=(c == 0), stop=(c == CC - 1))
                if part == 1:
                    copy_qk(ps, g, qsl)
                    del gen_live[(s, i)]
                    if g == 2:
                        # replicate q2 to partitions 64-127 so head2's QK
                        # operands share base 64 (DMA engines are idle)
                        nc.gpsimd.dma_start(q2b[D:P, qsl], qk8[2][0:D, qsl])
            else:
                sc = 4 * s + (i - 3)
                for c in crange:
                    nc.tensor.matmul(ps[:, :VW], xt_sb[c][:, P * sc:P * (sc + 1)],
                                     wv_sb[:, c, :],
                                     start=(c == 0), stop=(c == CC - 1))
                if part == 1:
                    copy_v(ps, sc)
                    del gen_live[(s, i)]

        def emit_gen_group(s, i, pool=None):
            emit_gen_half(s, i, 0, pool)
            emit_gen_half(s, i, 1, pool)

        tp_pending = []   # deferred (j, h, o_norm) so transposes don't stall
                          # the PE right behind their norm-mul

        # prologue: only what attention(0) heads 0/1 need up front; the
        # [q2|k2] group rides the first filler ticks so the engines start
        # exp work ~3us earlier
        for i in [0, 1, 3, 4, 5, 6]:
            emit_gen_group(0, i, pool=(ps_pj if i % 2 else None))

        for s in range(NQ):
            # Filler PE work sprinkled between kc chunks: next tile's gen and
            # the previous tile's projection. Gen items first (no deps), proj
            # items only emit once this tile's first tp flush has run (they
            # read ot written by the deferred transposes).
            gen_items = ([("gen", 0, 2, p) for p in range(2)] if s == 0 else []) \
                + ([("gen", s + 1, i, p) for i in range(7) for p in range(2)]
                   if s + 1 < NQ else [])
            proj_items = [("proj", s - 1, i, 0) for i in range(8)] if s > 0 else []
            filler = []
            while gen_items or proj_items:
                filler.extend(gen_items[:2]); del gen_items[:2]
                filler.extend(proj_items[:1]); del proj_items[:1]
            total_ticks = 3 * 2 * (s + 1)
            stride = max(1, total_ticks // (len(filler) + 1)) if filler else 10**9
            fill_state = {"tick": 0, "idx": 0, "proj_ok": False}

            def emit_filler_item(item):
                kind, a, b, p = item
                if kind == "gen":
                    emit_gen_half(a, b, p)
                else:
                    emit_proj_group(a, b)

            def filler_tick():
                fill_state["tick"] += 1
                while (fill_state["idx"] < len(filler)
                       and fill_state["tick"] >= stride * (fill_state["idx"] + 1)):
                    item = filler[fill_state["idx"]]
                    if item[0] == "proj" and not fill_state["proj_ok"]:
                        return
                    fill_state["idx"] += 1
                    emit_filler_item(item)

            def filler_drain():
                while fill_state["idx"] < len(filler):
                    item = filler[fill_state["idx"]]
                    fill_state["idx"] += 1
                    emit_filler_item(item)

            # ---- attention for q-tile j = s ----
            j = s
            nkc = 4 * (j + 1)
            for h in range(HL):
                if h < 2:
                    qt_, kt_, base = qk8[0], qk8[1], D * h
                else:
                    qt_, kt_, base = q2b, qk8[2], D
                rows = slice(base, base + D)
                # pv bank: qc=0 at kc=0 starts (lazily zeroes the region);
                # the other qc strips write-through via pending-zero.
                pv = ps_pv.tile([P, 4, P], F32, tag="pv", name=f"pv{j}_{h}")

                def emit_pv(pt1, kc, off):
                    for qc in range(off // P, 4):
                        nc.tensor.matmul(pv[:, qc, 0:65],
                                         pt1[:, P * qc:P * (qc + 1)],
                                         v_s[kc][:, 66 * h:66 * h + 65],
                                         start=(kc == 0 and qc == 0),
                                         stop=(kc == nkc - 1 and qc == 3),
                                         skip_group_check=True)

                pv_lag = []
                for kc in range(nkc):
                    diag = kc >= 4 * j
                    off = P * (kc - 4 * j) if diag else 0
                    st1 = ps_st.tile([P, QT], F32, tag="st",
                                     name=f"st{j}_{h}_{kc}")
                    pt1 = sb_pt.tile([P, QT], BF, tag="pt",
                                     name=f"pt{j}_{h}_{kc}")
                    lhsT = kt_[rows, None, P * kc:P * (kc + 1)].to_broadcast(
                        (D, 2, P))
                    rhs = qt_[rows, None,
                              QT * j + off:QT * (j + 1)].to_broadcast(
                        (D, 2, QT - off))
                    nc.tensor.matmul(st1[:, off:], lhsT, rhs,
                                     start=True, stop=not diag,
                                     perf_mode=DR)
                    if diag:
                        nc.tensor.matmul(st1[:, off:off + P],
                                         id_sb[:], mask_sb[:],
                                         start=False, stop=True,
                                         skip_group_check=True)
                    emit_exp_ap(pt1[:, off:], st1[:, off:], QT - off, j)
                    pv_lag.append((pt1, kc, off))
                    if len(pv_lag) > 5:
                        emit_pv(*pv_lag.pop(0))
                    if kc % 2 == 1:
                        filler_tick()
                while pv_lag:
                    emit_pv(*pv_lag.pop(0))
                # normalize: reciprocal of the per-q denominators (col 64)
                dn = sb_dn.tile([P, 4], F32, tag="dn", name=f"dn{j}_{h}")
                nc.vector.reciprocal(dn[:], pv[:, :, 64])
                bal.charge("dve", 4, "tt")
                on = sb_on.tile([P, 4, D], BF, tag="on", name=f"on{j}_{h}")
                nc.vector.tensor_tensor(on[:], pv[:, :, 0:D],
                                        dn[:, :, None].to_broadcast((P, 4, D)),
                                        MUL)
                bal.charge("dve", 4 * D, "tt")
                tp_pending.append((j, h, on))
                while len(tp_pending) > 1:
                    emit_tp(*tp_pending.pop(0))
                if h == 0:
                    fill_state["proj_ok"] = True
            filler_drain()

        # tail: flush remaining transposes, then the last row's projection
        # (alternating psum banks: gen's slots are free by now)
        while tp_pending:
            emit_tp(*tp_pending.pop(0))
        for i in range(8):
            emit_proj_group(NQ - 1, i, pool=(ps_gen if i % 2 == 0 else None))

    return nc


def prep_core_inputs(x, W_attn, b_attn, W_out, b, g):
    """Host-side shard prep for core (batch b, head group g)."""
    habs = [HL * g + h for h in range(HL)]
    wq = [W_attn[:, D * h:D * (h + 1)] for h in habs]
    wk = [W_attn[:, C + D * h:C + D * (h + 1)] for h in habs]
    wv_ = [W_attn[:, 2 * C + D * h:2 * C + D * (h + 1)] for h in habs]
    bq = [b_attn[D * h:D * (h + 1)] for h in habs]
    bk = [b_attn[C + D * h:C + D * (h + 1)] for h in habs]
    bvv = [b_attn[2 * C + D * h:2 * C + D * (h + 1)] for h in habs]

    # [q0|q1] [k0|k1] [q2|k2]; k-bias dropped entirely (it adds a per-row
    # constant to the scores, which softmax shift-invariance cancels)
    wqk = np.concatenate(
        [wq[0], wq[1], wk[0], wk[1], wq[2], wk[2]], axis=1)
    zz = np.zeros(D, dtype=np.float32)
    bqk = np.stack([
        np.concatenate([bq[0], bq[1]]),
        np.concatenate([zz, zz]),
        np.concatenate([bq[2], zz]),
    ], axis=1).astype(np.float32)

    wv_ext = np.zeros((C, VW), dtype=np.float32)
    bv_ext = np.zeros(VW, dtype=np.float32)
    for h in range(HL):
        wv_ext[:, 66 * h:66 * h + D] = wv_[h]
        bv_ext[66 * h:66 * h + D] = bvv[h]
        bv_ext[66 * h + D] = 1.0
    bv_tile = np.ascontiguousarray(
        np.broadcast_to(bv_ext, (P, VW))).astype(np.float32)

    wo_g = W_out[192 * g:192 * (g + 1), :]

    p = np.arange(P)
    # additive causal mask for the diagonal block: -600 where masked (l < p)
    mask = ((p[None, :] < p[:, None]) * -600.0).astype(BF_NP)

    return {
        "xT": np.ascontiguousarray(x[b].T).astype(BF_NP),
        "wqk": wqk.astype(BF_NP),
        "bqk": bqk,
        "wv": wv_ext.astype(BF_NP),
        "bv": bv_tile,
        "wo01": np.ascontiguousarray(wo_g[:P, :]).astype(BF_NP),
        "wo2": np.ascontiguousarray(wo_g[P:, :]).astype(BF_NP),
        "mask": mask,
        "ident": np.eye(P, dtype=BF_NP),
    }


_NC_CACHE = {}


def kernel(x, W_attn, b_attn, W_out, b_out):
    x = np.asarray(x, dtype=np.float32)
    W_attn = np.asarray(W_attn, dtype=np.float32)
    b_attn = np.asarray(b_attn, dtype=np.float32)
    W_out = np.asarray(W_out, dtype=np.float32)
    b_out = np.asarray(b_out, dtype=np.float32)
    B, n_seq, _ = x.shape

    if n_seq not in _NC_CACHE:
        _NC_CACHE[n_seq] = build_nc(n_seq)
    nc = _NC_CACHE[n_seq]

    in_maps = [prep_core_inputs(x, W_attn, b_attn, W_out, b, g)
               for b in range(B) for g in range(4)]
    res = bass_utils.run_bass_kernel_spmd(
        nc, in_maps, core_ids=list(range(N_CORES)))
    parts = [r["out"] for r in res.results]
    out = np.empty((B, n_seq, C), dtype=np.float32)
    for b in range(B):
        out[b] = parts[4 * b] + parts[4 * b + 1] + parts[4 * b + 2] \
            + parts[4 * b + 3] + b_out
    return out
